# revision 7
# baseline (speedup 1.0000x reference)
"""Causal self-attention, 8 TRN2 cores, head-parallel, zero input collectives.

Sharding: tensor-parallel over heads (2 heads/core) with x REPLICATED per
core (staged device-side, so no on-device AllGather), and the output
projection token-parallel: after attention, one small per-batch AllToAll
redistributes yT slices ([128 ch x 256 tok] per peer) so each core projects
only its own 256 tokens per batch against the FULL Wp, with bp folded into
the matmul via a 1-partition ones row. Collectives drop from 6 (2 AllGather
+ 4 ReduceScatter, ~12.5MB) to 2 AllToAlls (0.5MB each).

Schedule: batch-0 QKV -> batch-0 attention with batch-1 QKV/V-transpose
chains interleaved between score panels (fills the PE bubbles left by the
exp latency on ACT) -> batch-0 AllToAll (Pool, overlaps batch-1 attention)
-> batch-1 attention with batch-0 projection chains as late fillers ->
batch-1 AllToAll -> batch-1 projection. Attention emits score matmuls two
panels ahead of the PV accumulation (depth-2 software pipeline).

Per-core inputs:
  xt    [128, 8*8*512] bf16  xt[p, ((b*4+n)*8+co)*512+t] = x[b, n*512+t, co*128+p]
  wqkvt [1024, 384]    bf16  [Wq_i.T | Wk_i.T | Wv_i.T] (this core's 128 ch)
  bqkv  [1, 384]       f32   (q,k,v) interleaved per channel
  wpt   [1024, 1024]   bf16  full Wp.T
  bp    [1, 1024]      bf16  full bp
Output per core:
  out   [512, 1024]    bf16  row b*256+i = global (b, c*256+i)
"""

import sys

sys.path.insert(0, "/opt/trn_rl_repo")

import numpy as np
import ml_dtypes

import concourse.bass as bass
import concourse.mybir as mybir
import concourse.tile as tile
from concourse import bacc
from concourse.bass_utils import run_bass_kernel_spmd
from concourse.masks import make_identity

bf16 = ml_dtypes.bfloat16
B, T, C, H = 2, 2048, 1024, 16
HD = C // H              # 64
NCORE = 8
BT = B * T               # 4096
TPB = T // NCORE         # 256 tokens per core per batch (output shard)
HPC = H // NCORE         # 2 heads per core
CW = HPC * HD            # 128 channels per core
QB = 512                 # q-block width
NQB = T // QB            # 4 q-blocks per batch
KP = 128                 # k-panel width
SCALE = 1.0 / 8.0        # 1/sqrt(64)
CO = C // 128            # 8 contraction chunks
NBL = B * NQB            # 8 token blocks of 512
NDIAG = QB // KP         # 4 diagonal panels per q-block

f32 = mybir.dt.float32
bf = mybir.dt.bfloat16
AF = mybir.ActivationFunctionType
ALU = mybir.AluOpType
GROUP = [list(range(NCORE))]

_cached_nc = None


def _build(reps=1, phase="full"):
    do_qkv = phase in ("qkv", "attn", "a2a", "full")
    do_attn = phase in ("attn", "a2a", "full")
    do_a2a = phase in ("a2a", "full")
    do_proj = phase == "full"

    nc = bacc.Bacc("TRN2", target_bir_lowering=False, debug=False, num_devices=NCORE)
    xt_d = nc.dram_tensor("xt", [128, NBL * CO * QB], bf, kind="ExternalInput")
    wqkvt_d = nc.dram_tensor("wqkvt", [CO * 128, 3 * CW], bf, kind="ExternalInput")
    bqkv_d = nc.dram_tensor("bqkv", [1, 3 * CW], f32, kind="ExternalInput")
    wpt_d = nc.dram_tensor("wpt", [CO * 128, C], bf, kind="ExternalInput")
    bp_d = nc.dram_tensor("bp", [1, C], bf, kind="ExternalInput")
    out_d = nc.dram_tensor("out", [B * TPB, C], bf, kind="ExternalOutput")

    with tile.TileContext(nc) as tc:
        with tc.tile_pool(name="const", bufs=1) as cp, \
             tc.tile_pool(name="dram", bufs=1, space="DRAM") as dp, \
             tc.tile_pool(name="work", bufs=5) as wp, \
             tc.tile_pool(name="mm", bufs=2, space="PSUM") as mmp, \
             tc.tile_pool(name="stp", bufs=4, space="PSUM") as stp, \
             tc.tile_pool(name="otp", bufs=2, space="PSUM") as otp:

            # ---- persistent tiles ----
            xt = cp.tile([128, NBL, CO, QB], bf)       # full x, both batches
            wqkv = cp.tile([128, CO, 3 * CW], bf)
            bqkv = cp.tile([CW, 3], f32)
            wf = cp.tile([128, CO, C], bf)             # full Wp.T
            bp1 = cp.tile([1, C], bf)
            onesr = cp.tile([1, 128], bf)
            qT = cp.tile([128, BT], bf)
            kT = cp.tile([128, BT], bf)
            vT = cp.tile([128, BT], bf)
            vnat = cp.tile([128, B * HPC, T // KP, HD + 1], bf)
            yT = cp.tile([128, BT], bf)
            ytc = cp.tile([128, B, CO, TPB], bf)       # gathered y for own tokens
            ident = cp.tile([128, 128], bf)
            ones65 = cp.tile([HD + 1, HD], bf)
            # causal masks for the 4 diagonal-panel offsets, as column
            # slices of one [128, 896] tile: mask[p, u] = (u >= p + 384).
            # Diagonal panel joff uses cols [384-128*joff, 896-128*joff).
            maskw = QB + (NDIAG - 1) * KP
            maskbig = cp.tile([128, maskw], bf)

            # DRAM bounce buffers
            bq_bounce = dp.tile([1, 3 * CW], f32)
            a2a_in0 = dp.tile([NCORE * 128, TPB], bf)
            a2a_in1 = dp.tile([NCORE * 128, TPB], bf)
            a2a_out0 = dp.tile([NCORE * 128, TPB], bf)
            a2a_out1 = dp.tile([NCORE * 128, TPB], bf)
            a2a_ins = (a2a_in0, a2a_in1)
            a2a_outs = (a2a_out0, a2a_out1)

            if not do_qkv:
                nc.gpsimd.memset(qT[:], 0.0)
                nc.gpsimd.memset(kT[:], 0.0)
                nc.gpsimd.memset(vT[:], 0.0)
            if not do_attn:
                nc.gpsimd.memset(yT[:], 0.0)
                nc.gpsimd.memset(vnat[:], 0.0)
            if not do_a2a:
                nc.gpsimd.memset(ytc[:], 0.0)

            for _rep in range(reps):
                # ---- input DMAs (SP + ACT queues; earliest-needed first) ----
                nc.sync.dma_start(bq_bounce[:], bqkv_d.ap())
                nc.sync.dma_start(
                    bqkv[:],
                    bq_bounce[:].rearrange("o (p j) -> (o p) j", p=CW))
                nc.sync.dma_start(bp1[:], bp_d.ap())
                nc.scalar.dma_start(
                    wqkv[:],
                    wqkvt_d.ap().rearrange("(co p) j -> p co j", p=128))
                for bn in range(NBL):
                    eng = nc.sync if bn % 2 == 0 else nc.scalar
                    src = xt_d.ap()[:, bn * CO * QB:(bn + 1) * CO * QB]
                    eng.dma_start(xt[:, bn], src.rearrange("p (co t) -> p co t",
                                                           co=CO))
                nc.sync.dma_start(
                    wf[:], wpt_d.ap().rearrange("(co p) j -> p co j", p=128))

                # ---- const setup (Pool queue; idle until the AllToAlls) ----
                make_identity(nc, ident[:])
                nc.gpsimd.memset(ones65[:], 1.0)
                nc.gpsimd.memset(onesr[:], 1.0)
                nc.gpsimd.memset(maskbig[:], 1.0)
                nc.gpsimd.affine_select(
                    out=maskbig[:], in_=maskbig[:],
                    compare_op=ALU.is_ge, fill=0.0,
                    base=-(NDIAG - 1) * KP, channel_multiplier=-1,
                    pattern=[[1, maskw]],
                )
                if do_attn:
                    nc.gpsimd.memset(vnat[:, :, :, HD:HD + 1], 1.0)

                dsts = (qT, kT, vT)

                def qkv_chain(b, n, p):
                    # one projection chunk: [128 out-ch, 512 tok]
                    tok = b * T + n * QB
                    ps = mmp.tile([128, QB], f32, tag="mm")
                    for co in range(CO):
                        nc.tensor.matmul(
                            ps[:],
                            wqkv[:, co, p * CW:(p + 1) * CW],
                            xt[:, b * NQB + n, co, :],
                            start=(co == 0),
                            stop=(co == CO - 1),
                        )
                    nc.vector.tensor_add(
                        dsts[p][:, tok:tok + QB],
                        ps[:],
                        bqkv[:, p:p + 1].to_broadcast((128, QB)),
                    )

                def vtrans_group(b, n, h):
                    # natural-layout V panels for block n, head h
                    for kc in range(n * NDIAG, (n + 1) * NDIAG):
                        tp = mmp.tile([128, HD], bf, tag="mm")
                        nc.tensor.transpose(
                            tp[:],
                            vT[HD * h:HD * (h + 1),
                               b * T + kc * KP: b * T + (kc + 1) * KP],
                            ident[HD * h:HD * (h + 1), HD * h:HD * (h + 1)],
                        )
                        nc.vector.tensor_copy(
                            vnat[:, b * HPC + h, kc, 0:HD], tp[:])

                def attn_block(b, qb, fillers, fill_every=2):
                    # depth-2 software pipeline: emit score panels (st+exp+
                    # mask) two j-steps ahead of the PV accumulations, and
                    # pull one PE filler chain every `fill_every` j-steps.
                    n_kp = (qb + 1) * NDIAG
                    q_sl = slice(b * T + qb * QB, b * T + (qb + 1) * QB)
                    ots = [otp.tile([HD + 1, QB], f32, tag="ot",
                                    name=f"ot_{b}_{qb}_{h}")
                           for h in range(HPC)]
                    pts = {}

                    def emit_st(j):
                        k_sl = slice(b * T + j * KP, b * T + (j + 1) * KP)
                        for h in range(HPC):
                            hsl = slice(HD * h, HD * (h + 1))
                            st = stp.tile([128, QB], f32, tag="st")
                            nc.tensor.matmul(
                                st[:], kT[hsl, k_sl], qT[hsl, q_sl],
                                start=True, stop=True,
                            )
                            pt = wp.tile([128, QB], bf, tag="pt")
                            nc.scalar.activation(pt[:], st[:], AF.Exp,
                                                 scale=SCALE)
                            joff = j - qb * NDIAG
                            if joff >= 0:
                                moff = (NDIAG - 1 - joff) * KP
                                nc.vector.tensor_mul(
                                    pt[:], pt[:],
                                    maskbig[:, moff:moff + QB])
                            pts[(j, h)] = pt

                    def emit_pv(j):
                        for h in range(HPC):
                            nc.tensor.matmul(
                                ots[h][:],
                                vnat[:, b * HPC + h, j, :],
                                pts.pop((j, h))[:],
                                start=(j == 0),
                                stop=(j == n_kp - 1),
                            )

                    for j in range(n_kp + 1):
                        if j < n_kp:
                            emit_st(j)
                        if j >= 1:
                            emit_pv(j - 1)
                        if j % fill_every == 0:
                            for f in fillers:
                                f()
                                break

                    # normalize by softmax denominators (last PV row)
                    for h in range(HPC):
                        rec = wp.tile([HD + 1, QB], bf, tag="rec",
                                      name=f"rec_{b}_{qb}_{h}")
                        with nc.allow_low_precision(
                                reason="bf16 denominator broadcast"):
                            nc.vector.reciprocal(
                                rec[HD:HD + 1, :], ots[h][HD:HD + 1, :])
                        ocp = wp.tile([HD, QB], f32, tag="ocp",
                                      name=f"ocp_{b}_{qb}_{h}")
                        nc.vector.tensor_copy(ocp[:], ots[h][0:HD, :])
                        # broadcast the reciprocal over 64 partitions (PE)
                        bc = otp.tile([HD, QB], f32, tag="ot",
                                      name=f"bc_{b}_{qb}_{h}")
                        nc.tensor.matmul(
                            bc[:],
                            ones65[HD:HD + 1, :],
                            rec[HD:HD + 1, :],
                            start=True, stop=True,
                        )
                        if h == 0:
                            nc.vector.tensor_mul(
                                yT[0:HD, q_sl], ocp[:], bc[:])
                        else:
                            t64 = wp.tile([HD, QB], bf, tag="t64")
                            nc.vector.tensor_mul(t64[:], ocp[:], bc[:])
                            nc.sync.dma_start(yT[HD:2 * HD, q_sl], t64[:])

                def a2a(b):
                    # yT[:, batch b] -> per-peer [128 x 256] slices ->
                    # AllToAll -> ytc[:, b] = y[all 1024 ch, own 256 tok]
                    nc.gpsimd.dma_start(
                        a2a_ins[b][:].rearrange("(d p) t -> p d t", p=128),
                        yT[:, b * T:(b + 1) * T].rearrange(
                            "p (d t) -> p d t", d=NCORE))
                    nc.gpsimd.collective_compute(
                        "AllToAll", ALU.bypass, replica_groups=GROUP,
                        ins=[a2a_ins[b][:].opt()],
                        outs=[a2a_outs[b][:].opt()],
                    )
                    nc.gpsimd.dma_start(
                        ytc[:, b],
                        a2a_outs[b][:].rearrange("(d p) t -> p d t", p=128))

                def proj_chain(b, rb):
                    # out rows [b*256 + rb*128, +128) = ytc.T @ Wp.T + bp
                    osb = wp.tile([128, C], bf, tag="osb")
                    for half in range(C // QB):
                        ps = mmp.tile([128, QB], f32, tag="mm")
                        # bias first (1-partition ones row), then accumulate
                        nc.tensor.matmul(
                            ps[:], onesr[:],
                            bp1[:, half * QB:(half + 1) * QB],
                            start=True, stop=False,
                        )
                        for co in range(CO):
                            nc.tensor.matmul(
                                ps[:],
                                ytc[:, b, co, rb * 128:(rb + 1) * 128],
                                wf[:, co, half * QB:(half + 1) * QB],
                                start=False,
                                stop=(co == CO - 1),
                            )
                        nc.vector.tensor_copy(
                            osb[:, half * QB:(half + 1) * QB], ps[:])
                    nc.sync.dma_start(
                        out_d.ap()[b * TPB + rb * 128:
                                   b * TPB + (rb + 1) * 128, :],
                        osb[:])

                # ---------------- schedule ----------------
                if do_qkv:
                    for n in range(NQB):
                        for p in range(3):
                            qkv_chain(0, n, p)
                        if do_attn:
                            for h in range(HPC):
                                vtrans_group(0, n, h)

                def b1_fillers():
                    if do_qkv:
                        for n in range(NQB):
                            for p in range(3):
                                yield lambda n=n, p=p: qkv_chain(1, n, p)
                            if do_attn:
                                for h in range(HPC):
                                    yield lambda n=n, h=h: vtrans_group(1, n, h)

                def b0_proj_fillers():
                    if do_proj:
                        for rb in range(TPB // 128):
                            yield lambda rb=rb: proj_chain(0, rb)

                f1 = b1_fillers()
                if do_attn:
                    for qb in range(NQB):
                        attn_block(0, qb, f1)
                for f in f1:  # drain any leftovers
                    f()
                if do_a2a:
                    a2a(0)

                f0 = b0_proj_fillers()
                if do_attn:
                    for qb in range(NQB):
                        # only pull proj fillers once the AllToAll result
                        # has plausibly landed (last q-block of b1 attn)
                        attn_block(1, qb, f0 if qb >= 3 else iter(()),
                                   fill_every=4)
                for f in f0:
                    f()
                if do_a2a:
                    a2a(1)
                if do_proj:
                    for rb in range(TPB // 128):
                        proj_chain(1, rb)

    nc.finalize()
    return nc


def _prep_in_maps(x, Wq, bq, Wk, bk, Wv, bv, Wp, bp):
    # xt[p, ((b*4+n)*8+co)*512 + t] = x[b, n*512+t, co*128+p], replicated
    x2 = x.astype(bf16).reshape(B, NQB, QB, CO, 128)
    xt = np.ascontiguousarray(x2.transpose(4, 0, 1, 3, 2))
    xt = xt.reshape(128, NBL * CO * QB)
    wpt = np.ascontiguousarray(Wp.T).astype(bf16)          # [1024, 1024]
    bp1 = np.ascontiguousarray(bp.astype(bf16).reshape(1, C))
    in_maps = []
    for i in range(NCORE):
        ch = slice(CW * i, CW * (i + 1))
        wqkvt = np.ascontiguousarray(np.concatenate(
            [Wq[ch].T, Wk[ch].T, Wv[ch].T], axis=1).astype(bf16))  # [C, 384]
        bqkv = np.stack([bq[ch], bk[ch], bv[ch]],
                        axis=1).astype(np.float32).reshape(1, 3 * CW)
        in_maps.append({
            "xt": xt,
            "wqkvt": wqkvt,
            "bqkv": np.ascontiguousarray(bqkv),
            "wpt": wpt,
            "bp": bp1,
        })
    return in_maps


def _assemble(results):
    # results[c]["out"]: [512, 1024] bf16, row b*256+i = global (b, c*256+i)
    out = np.empty((B, NCORE, TPB, C), np.float32)
    for c, r in enumerate(results):
        out[:, c] = np.asarray(r["out"]).reshape(B, TPB, C)
    return out.reshape(B, T, C)


def kernel(x, Wq, bq, Wk, bk, Wv, bv, Wp, bp):
    global _cached_nc
    x = np.asarray(x, np.float32)
    Wq, bq = np.asarray(Wq, np.float32), np.asarray(bq, np.float32)
    Wk, bk = np.asarray(Wk, np.float32), np.asarray(bk, np.float32)
    Wv, bv = np.asarray(Wv, np.float32), np.asarray(bv, np.float32)
    Wp, bp = np.asarray(Wp, np.float32), np.asarray(bp, np.float32)

    if _cached_nc is None:
        _cached_nc = _build()
    nc = _cached_nc

    in_maps = _prep_in_maps(x, Wq, bq, Wk, bk, Wv, bv, Wp, bp)
    res = run_bass_kernel_spmd(nc, in_maps, core_ids=list(range(NCORE)))
    return _assemble(res.results)


# revision 13
# speedup vs baseline: 2.7455x; 2.7455x over previous
"""Causal self-attention, 8 TRN2 cores, head-parallel, zero input collectives.

Sharding: tensor-parallel over heads (2 heads/core) with x REPLICATED per
core (staged device-side, so no on-device AllGather), and the output
projection token-parallel: after attention, one small per-batch AllToAll
redistributes yT slices ([128 ch x 256 tok] per peer) so each core projects
only its own 256 tokens per batch against the FULL Wp, with bp folded into
the matmul via a 1-partition ones row. Collectives drop from 6 (2 AllGather
+ 4 ReduceScatter, ~12.5MB) to 2 AllToAlls (0.5MB each).

Schedule: batch-0 QKV -> batch-0 attention with batch-1 QKV/V-transpose
chains interleaved between score panels (fills the PE bubbles left by the
exp latency on ACT) -> batch-0 AllToAll (Pool, overlaps batch-1 attention)
-> batch-1 attention with batch-0 projection chains as late fillers ->
batch-1 AllToAll -> batch-1 projection. Attention emits score matmuls two
panels ahead of the PV accumulation (depth-2 software pipeline).

Per-core inputs:
  xt    [128, 8*8*512] bf16  xt[p, ((b*4+n)*8+co)*512+t] = x[b, n*512+t, co*128+p]
  wqkvt [1024, 384]    bf16  [Wq_i.T | Wk_i.T | Wv_i.T] (this core's 128 ch)
  bqkv  [1, 384]       f32   (q,k,v) interleaved per channel
  wpt   [1024, 1024]   bf16  full Wp.T
  bp    [1, 1024]      bf16  full bp
Output per core:
  out   [512, 1024]    bf16  row b*256+i = global (b, c*256+i)
"""

import sys

sys.path.insert(0, "/opt/trn_rl_repo")

import numpy as np
import ml_dtypes

import concourse.bass as bass
import concourse.mybir as mybir
import concourse.tile as tile
from concourse import bacc
from concourse.bass_utils import run_bass_kernel_spmd
from concourse.masks import make_identity

bf16 = ml_dtypes.bfloat16
B, T, C, H = 2, 2048, 1024, 16
HD = C // H              # 64
NCORE = 8
BT = B * T               # 4096
TPB = T // NCORE         # 256 tokens per core per batch (output shard)
HPC = H // NCORE         # 2 heads per core
CW = HPC * HD            # 128 channels per core
QB = 512                 # q-block width
NQB = T // QB            # 4 q-blocks per batch
KP = 128                 # k-panel width
SCALE = 1.0 / 8.0        # 1/sqrt(64)
CO = C // 128            # 8 contraction chunks
NBL = B * NQB            # 8 token blocks of 512
NDIAG = QB // KP         # 4 diagonal panels per q-block

f32 = mybir.dt.float32
bf = mybir.dt.bfloat16
AF = mybir.ActivationFunctionType
ALU = mybir.AluOpType
GROUP = [list(range(NCORE))]

_cached_nc = None


def _build(reps=1, phase="full"):
    do_qkv = phase in ("qkv", "attn", "ath1", "a2a", "full")
    do_attn = phase in ("attn", "ath1", "a2a", "full")
    do_a2a = phase in ("a2a", "full")
    do_proj = phase == "full"
    n_cc = int(phase[2:]) if phase.startswith("cc") else 0
    hpc_eff = 1 if phase == "ath1" else HPC

    nc = bacc.Bacc("TRN2", target_bir_lowering=False, debug=False, num_devices=NCORE)
    xt_d = nc.dram_tensor("xt", [128, NBL * CO * QB], bf, kind="ExternalInput")
    wqkvt_d = nc.dram_tensor("wqkvt", [CO * 128, 3 * CW], bf, kind="ExternalInput")
    bqkv_d = nc.dram_tensor("bqkv", [1, 3 * CW], f32, kind="ExternalInput")
    wpt_d = nc.dram_tensor("wpt", [CO * 128, C], bf, kind="ExternalInput")
    bp_d = nc.dram_tensor("bp", [1, C], bf, kind="ExternalInput")
    out_d = nc.dram_tensor("out", [B * TPB, C], bf, kind="ExternalOutput")

    with tile.TileContext(nc) as tc:
        with tc.tile_pool(name="const", bufs=1) as cp, \
             tc.tile_pool(name="dram", bufs=1, space="DRAM") as dp, \
             tc.tile_pool(name="work", bufs=5) as wp, \
             tc.tile_pool(name="mm", bufs=2, space="PSUM") as mmp, \
             tc.tile_pool(name="stp", bufs=4, space="PSUM") as stp, \
             tc.tile_pool(name="otp", bufs=2, space="PSUM") as otp:

            # ---- persistent tiles ----
            xt = cp.tile([128, NBL, CO, QB], bf)       # full x, both batches
            wqkv = cp.tile([128, CO, 3 * CW], bf)
            bqkv = cp.tile([CW, 3], f32)
            wf = cp.tile([128, CO, C], bf)             # full Wp.T
            bp1 = cp.tile([1, C], bf)
            onesr = cp.tile([1, 128], bf)
            qT = cp.tile([128, BT], bf)
            kT = cp.tile([128, BT], bf)
            vT = cp.tile([128, BT], bf)
            vnat = cp.tile([128, B * HPC, T // KP, HD + 1], bf)
            yT = cp.tile([128, BT], bf)
            ytc = cp.tile([128, B, CO, TPB], bf)       # gathered y for own tokens
            ident = cp.tile([128, 128], bf)
            ones65 = cp.tile([HD + 1, HD], bf)
            # causal-mask factors: st[:, diag block] += Lt.T @ negI adds
            # -240 above the diagonal (exp(-240/8 + s) ~ 0), so no separate
            # mask multiply sits on the st->exp->pv dependency chain.
            lt = cp.tile([128, 128], bf)     # lt[d, p] = (d < p)
            negi = cp.tile([128, QB], bf)    # [-240*I | zeros]

            # DRAM bounce buffers
            bq_bounce = dp.tile([1, 3 * CW], f32)
            a2a_in0 = dp.tile([NCORE * 128, TPB], bf)
            a2a_in1 = dp.tile([NCORE * 128, TPB], bf)
            a2a_out0 = dp.tile([NCORE * 128, TPB], bf)
            a2a_out1 = dp.tile([NCORE * 128, TPB], bf)
            a2a_ins = (a2a_in0, a2a_in1)
            a2a_outs = (a2a_out0, a2a_out1)

            if not do_qkv:
                nc.gpsimd.memset(qT[:], 0.0)
                nc.gpsimd.memset(kT[:], 0.0)
                nc.gpsimd.memset(vT[:], 0.0)
            if not do_attn:
                nc.gpsimd.memset(yT[:], 0.0)
                nc.gpsimd.memset(vnat[:], 0.0)
            if not do_a2a:
                nc.gpsimd.memset(ytc[:], 0.0)

            for _rep in range(reps):
                if n_cc:
                    # collective micro-benchmark: n_cc AllToAlls per rep
                    for i in range(n_cc):
                        nc.gpsimd.collective_compute(
                            "AllToAll", ALU.bypass, replica_groups=GROUP,
                            ins=[a2a_ins[i % 2][:].opt()],
                            outs=[a2a_outs[i % 2][:].opt()],
                        )
                    continue
                # ---- input DMAs (SP + ACT queues; earliest-needed first) ----
                nc.sync.dma_start(bq_bounce[:], bqkv_d.ap())
                nc.sync.dma_start(
                    bqkv[:],
                    bq_bounce[:].rearrange("o (p j) -> (o p) j", p=CW))
                nc.sync.dma_start(bp1[:], bp_d.ap())
                nc.scalar.dma_start(
                    wqkv[:],
                    wqkvt_d.ap().rearrange("(co p) j -> p co j", p=128))
                for bn in range(NBL):
                    eng = nc.sync if bn % 2 == 0 else nc.scalar
                    src = xt_d.ap()[:, bn * CO * QB:(bn + 1) * CO * QB]
                    eng.dma_start(xt[:, bn], src.rearrange("p (co t) -> p co t",
                                                           co=CO))
                nc.sync.dma_start(
                    wf[:], wpt_d.ap().rearrange("(co p) j -> p co j", p=128))

                # ---- const setup (Pool queue; idle until the AllToAlls) ----
                make_identity(nc, ident[:])
                nc.gpsimd.memset(ones65[:], 1.0)
                nc.gpsimd.memset(onesr[:], 1.0)
                nc.gpsimd.memset(lt[:], 1.0)
                nc.gpsimd.affine_select(
                    out=lt[:], in_=lt[:],
                    compare_op=ALU.is_ge, fill=0.0,
                    base=-1, channel_multiplier=-1,
                    pattern=[[1, 128]],
                )
                nc.gpsimd.memset(negi[:, 128:], 0.0)
                nc.scalar.activation(negi[:, 0:128], ident[:], AF.Copy,
                                     scale=-240.0)
                if do_attn:
                    nc.gpsimd.memset(vnat[:, :, :, HD:HD + 1], 1.0)

                dsts = (qT, kT, vT)

                def qkv_chain(b, n, p):
                    # one projection chunk: [128 out-ch, 512 tok]
                    tok = b * T + n * QB
                    ps = mmp.tile([128, QB], f32, tag="mm")
                    for co in range(CO):
                        nc.tensor.matmul(
                            ps[:],
                            wqkv[:, co, p * CW:(p + 1) * CW],
                            xt[:, b * NQB + n, co, :],
                            start=(co == 0),
                            stop=(co == CO - 1),
                        )
                    nc.vector.tensor_add(
                        dsts[p][:, tok:tok + QB],
                        ps[:],
                        bqkv[:, p:p + 1].to_broadcast((128, QB)),
                    )

                def vtrans_group(b, n, h):
                    # natural-layout V panels for block n, head h
                    for kc in range(n * NDIAG, (n + 1) * NDIAG):
                        tp = mmp.tile([128, HD], bf, tag="mm")
                        nc.tensor.transpose(
                            tp[:],
                            vT[HD * h:HD * (h + 1),
                               b * T + kc * KP: b * T + (kc + 1) * KP],
                            ident[HD * h:HD * (h + 1), HD * h:HD * (h + 1)],
                        )
                        nc.vector.tensor_copy(
                            vnat[:, b * HPC + h, kc, 0:HD], tp[:])

                def attn_block(b, qb, fillers, fill_every=2):
                    # depth-2 software pipeline: emit score panels (st+exp+
                    # mask) two j-steps ahead of the PV accumulations, and
                    # pull one PE filler chain every `fill_every` j-steps.
                    n_kp = (qb + 1) * NDIAG
                    q_sl = slice(b * T + qb * QB, b * T + (qb + 1) * QB)
                    ots = [otp.tile([HD + 1, QB], f32, tag="ot",
                                    name=f"ot_{b}_{qb}_{h}")
                           for h in range(HPC)]
                    pts = {}

                    def emit_st(j):
                        k_sl = slice(b * T + j * KP, b * T + (j + 1) * KP)
                        joff = j - qb * NDIAG
                        # columns < joff*KP are entirely above the causal
                        # diagonal: skip them in exp and PV
                        u0 = max(joff, 0) * KP
                        for h in range(hpc_eff):
                            hsl = slice(HD * h, HD * (h + 1))
                            st = stp.tile([128, QB], f32, tag="st")
                            nc.tensor.matmul(
                                st[:], kT[hsl, k_sl], qT[hsl, q_sl],
                                start=True, stop=(joff < 0),
                            )
                            if joff >= 0:
                                nc.tensor.matmul(
                                    st[:, u0:], lt[:], negi[:, :QB - u0],
                                    start=False, stop=True,
                                )
                            pt = wp.tile([128, QB], bf, tag="pt")
                            nc.scalar.activation(pt[:, u0:], st[:, u0:],
                                                 AF.Exp, scale=SCALE)
                            pts[(j, h)] = (pt, u0)

                    def emit_pv(j):
                        for h in range(hpc_eff):
                            pt, u0 = pts.pop((j, h))
                            nc.tensor.matmul(
                                ots[h][:, u0:],
                                vnat[:, b * HPC + h, j, :],
                                pt[:, u0:],
                                start=(j == 0),
                                stop=(j == n_kp - 1),
                            )

                    for j in range(n_kp + 1):
                        if j < n_kp:
                            emit_st(j)
                        if j >= 1:
                            emit_pv(j - 1)
                        if j % fill_every == 0:
                            for f in fillers:
                                f()
                                break

                    # normalize by softmax denominators (last PV row)
                    for h in range(hpc_eff):
                        rec = wp.tile([HD + 1, QB], bf, tag="rec",
                                      name=f"rec_{b}_{qb}_{h}")
                        with nc.allow_low_precision(
                                reason="bf16 denominator broadcast"):
                            nc.vector.reciprocal(
                                rec[HD:HD + 1, :], ots[h][HD:HD + 1, :])
                        ocp = wp.tile([HD, QB], f32, tag="ocp",
                                      name=f"ocp_{b}_{qb}_{h}")
                        nc.vector.tensor_copy(ocp[:], ots[h][0:HD, :])
                        # broadcast the reciprocal over 64 partitions (PE)
                        bc = otp.tile([HD, QB], f32, tag="ot",
                                      name=f"bc_{b}_{qb}_{h}")
                        nc.tensor.matmul(
                            bc[:],
                            ones65[HD:HD + 1, :],
                            rec[HD:HD + 1, :],
                            start=True, stop=True,
                        )
                        if h == 0:
                            nc.vector.tensor_mul(
                                yT[0:HD, q_sl], ocp[:], bc[:])
                        else:
                            t64 = wp.tile([HD, QB], bf, tag="t64")
                            nc.vector.tensor_mul(t64[:], ocp[:], bc[:])
                            nc.sync.dma_start(yT[HD:2 * HD, q_sl], t64[:])

                def a2a(b):
                    # yT[:, batch b] -> per-peer [128 x 256] slices ->
                    # AllToAll -> ytc[:, b] = y[all 1024 ch, own 256 tok]
                    nc.gpsimd.dma_start(
                        a2a_ins[b][:].rearrange("(d p) t -> p d t", p=128),
                        yT[:, b * T:(b + 1) * T].rearrange(
                            "p (d t) -> p d t", d=NCORE))
                    nc.gpsimd.collective_compute(
                        "AllToAll", ALU.bypass, replica_groups=GROUP,
                        ins=[a2a_ins[b][:].opt()],
                        outs=[a2a_outs[b][:].opt()],
                    )
                    nc.gpsimd.dma_start(
                        ytc[:, b],
                        a2a_outs[b][:].rearrange("(d p) t -> p d t", p=128))

                def proj_chain(b, rb):
                    # out rows [b*256 + rb*128, +128) = ytc.T @ Wp.T + bp
                    osb = wp.tile([128, C], bf, tag="osb")
                    for half in range(C // QB):
                        ps = mmp.tile([128, QB], f32, tag="mm")
                        # bias first (1-partition ones row), then accumulate
                        nc.tensor.matmul(
                            ps[:], onesr[:],
                            bp1[:, half * QB:(half + 1) * QB],
                            start=True, stop=False,
                        )
                        for co in range(CO):
                            nc.tensor.matmul(
                                ps[:],
                                ytc[:, b, co, rb * 128:(rb + 1) * 128],
                                wf[:, co, half * QB:(half + 1) * QB],
                                start=False,
                                stop=(co == CO - 1),
                            )
                        nc.vector.tensor_copy(
                            osb[:, half * QB:(half + 1) * QB], ps[:])
                    nc.sync.dma_start(
                        out_d.ap()[b * TPB + rb * 128:
                                   b * TPB + (rb + 1) * 128, :],
                        osb[:])

                # ---------------- schedule ----------------
                if do_qkv:
                    for n in range(NQB):
                        for p in range(3):
                            qkv_chain(0, n, p)
                        if do_attn:
                            for h in range(hpc_eff):
                                vtrans_group(0, n, h)

                def b1_fillers():
                    if do_qkv:
                        for n in range(NQB):
                            for p in range(3):
                                yield lambda n=n, p=p: qkv_chain(1, n, p)
                            if do_attn:
                                for h in range(hpc_eff):
                                    yield lambda n=n, h=h: vtrans_group(1, n, h)

                f1 = b1_fillers()
                if do_attn:
                    for qb in range(NQB):
                        attn_block(0, qb, f1)
                for f in f1:  # drain any leftovers
                    f()
                if do_a2a:
                    a2a(0)

                if do_attn:
                    for qb in range(NQB):
                        attn_block(1, qb, iter(()))
                if do_proj:
                    for rb in range(TPB // 128):
                        proj_chain(0, rb)
                if do_a2a:
                    a2a(1)
                if do_proj:
                    for rb in range(TPB // 128):
                        proj_chain(1, rb)

    nc.finalize()
    return nc


def _prep_in_maps(x, Wq, bq, Wk, bk, Wv, bv, Wp, bp):
    # xt[p, ((b*4+n)*8+co)*512 + t] = x[b, n*512+t, co*128+p], replicated
    x2 = x.astype(bf16).reshape(B, NQB, QB, CO, 128)
    xt = np.ascontiguousarray(x2.transpose(4, 0, 1, 3, 2))
    xt = xt.reshape(128, NBL * CO * QB)
    wpt = np.ascontiguousarray(Wp.T).astype(bf16)          # [1024, 1024]
    bp1 = np.ascontiguousarray(bp.astype(bf16).reshape(1, C))
    in_maps = []
    for i in range(NCORE):
        ch = slice(CW * i, CW * (i + 1))
        wqkvt = np.ascontiguousarray(np.concatenate(
            [Wq[ch].T, Wk[ch].T, Wv[ch].T], axis=1).astype(bf16))  # [C, 384]
        bqkv = np.stack([bq[ch], bk[ch], bv[ch]],
                        axis=1).astype(np.float32).reshape(1, 3 * CW)
        in_maps.append({
            "xt": xt,
            "wqkvt": wqkvt,
            "bqkv": np.ascontiguousarray(bqkv),
            "wpt": wpt,
            "bp": bp1,
        })
    return in_maps


def _assemble(results):
    # results[c]["out"]: [512, 1024] bf16, row b*256+i = global (b, c*256+i)
    out = np.empty((B, NCORE, TPB, C), np.float32)
    for c, r in enumerate(results):
        out[:, c] = np.asarray(r["out"]).reshape(B, TPB, C)
    return out.reshape(B, T, C)


def kernel(x, Wq, bq, Wk, bk, Wv, bv, Wp, bp):
    global _cached_nc
    x = np.asarray(x, np.float32)
    Wq, bq = np.asarray(Wq, np.float32), np.asarray(bq, np.float32)
    Wk, bk = np.asarray(Wk, np.float32), np.asarray(bk, np.float32)
    Wv, bv = np.asarray(Wv, np.float32), np.asarray(bv, np.float32)
    Wp, bp = np.asarray(Wp, np.float32), np.asarray(bp, np.float32)

    if _cached_nc is None:
        _cached_nc = _build()
    nc = _cached_nc

    in_maps = _prep_in_maps(x, Wq, bq, Wk, bk, Wv, bv, Wp, bp)
    res = run_bass_kernel_spmd(nc, in_maps, core_ids=list(range(NCORE)))
    return _assemble(res.results)


# revision 17
# speedup vs baseline: 2.8283x; 1.0302x over previous
"""Causal self-attention, 8 TRN2 cores, head-parallel, zero input collectives.

Sharding: tensor-parallel over heads (2 heads/core) with x REPLICATED per
core (staged device-side, so no on-device AllGather), and the output
projection token-parallel: after attention, one small per-batch AllToAll
redistributes yT slices ([128 ch x 256 tok] per peer) so each core projects
only its own 256 tokens per batch against the FULL Wp, with bp folded into
the matmul via a 1-partition ones row. Collectives drop from 6 (2 AllGather
+ 4 ReduceScatter, ~12.5MB) to 2 AllToAlls (0.5MB each).

Schedule: batch-0 QKV -> batch-0 attention with batch-1 QKV/V-transpose
chains interleaved between score panels (fills the PE bubbles left by the
exp latency on ACT) -> batch-0 AllToAll (Pool, overlaps batch-1 attention)
-> batch-1 attention with batch-0 projection chains as late fillers ->
batch-1 AllToAll -> batch-1 projection. Attention emits score matmuls two
panels ahead of the PV accumulation (depth-2 software pipeline).

Per-core inputs:
  xt    [128, 8*8*512] bf16  xt[p, ((b*4+n)*8+co)*512+t] = x[b, n*512+t, co*128+p]
  wqkvt [1024, 384]    bf16  [Wq_i.T | Wk_i.T | Wv_i.T] (this core's 128 ch)
  bqkv  [1, 384]       f32   (q,k,v) interleaved per channel
  wpt   [1024, 1024]   bf16  full Wp.T
  bp    [1, 1024]      bf16  full bp
Output per core:
  out   [512, 1024]    bf16  row b*256+i = global (b, c*256+i)
"""

import sys

sys.path.insert(0, "/opt/trn_rl_repo")

import numpy as np
import ml_dtypes

import concourse.bass as bass
import concourse.mybir as mybir
import concourse.tile as tile
from concourse import bacc
from concourse.bass_utils import run_bass_kernel_spmd
from concourse.masks import make_identity

bf16 = ml_dtypes.bfloat16
B, T, C, H = 2, 2048, 1024, 16
HD = C // H              # 64
NCORE = 8
BT = B * T               # 4096
TPB = T // NCORE         # 256 tokens per core per batch (output shard)
HPC = H // NCORE         # 2 heads per core
CW = HPC * HD            # 128 channels per core
QB = 512                 # q-block width
NQB = T // QB            # 4 q-blocks per batch
KP = 128                 # k-panel width
SCALE = 1.0 / 8.0        # 1/sqrt(64)
CO = C // 128            # 8 contraction chunks
NBL = B * NQB            # 8 token blocks of 512
NDIAG = QB // KP         # 4 diagonal panels per q-block

f32 = mybir.dt.float32
bf = mybir.dt.bfloat16
AF = mybir.ActivationFunctionType
ALU = mybir.AluOpType
GROUP = [list(range(NCORE))]

_cached_nc = None


def _build(reps=1, phase="full"):
    do_qkv = phase in ("qkv", "attn", "ath1", "atnx", "atsp", "a2a", "full")
    do_attn = phase in ("attn", "ath1", "atnx", "atsp", "a2a", "full")
    do_a2a = phase in ("a2a", "full")
    do_proj = phase == "full"
    n_cc = int(phase[2:]) if phase.startswith("cc") else 0
    hpc_eff = 1 if phase == "ath1" else HPC
    no_exp = phase == "atnx"
    split_exp = phase == "atsp"

    nc = bacc.Bacc("TRN2", target_bir_lowering=False, debug=False, num_devices=NCORE)
    xt_d = nc.dram_tensor("xt", [128, NBL * CO * QB], bf, kind="ExternalInput")
    wqkvt_d = nc.dram_tensor("wqkvt", [CO * 128, 3 * CW], bf, kind="ExternalInput")
    bqkv_d = nc.dram_tensor("bqkv", [1, 3 * CW], f32, kind="ExternalInput")
    wpt_d = nc.dram_tensor("wpt", [CO * 128, C], bf, kind="ExternalInput")
    bp_d = nc.dram_tensor("bp", [1, C], bf, kind="ExternalInput")
    out_d = nc.dram_tensor("out", [B * TPB, C], bf, kind="ExternalOutput")

    with tile.TileContext(nc) as tc:
        with tc.tile_pool(name="const", bufs=1) as cp, \
             tc.tile_pool(name="dram", bufs=1, space="DRAM") as dp, \
             tc.tile_pool(name="work", bufs=5) as wp, \
             tc.tile_pool(name="mm", bufs=2, space="PSUM") as mmp, \
             tc.tile_pool(name="stp", bufs=4, space="PSUM") as stp, \
             tc.tile_pool(name="otp", bufs=2, space="PSUM") as otp:

            # ---- persistent tiles ----
            xt = cp.tile([128, NBL, CO, QB], bf)       # full x, both batches
            wqkv = cp.tile([128, CO, 3 * CW], bf)
            bqkv = cp.tile([CW, 3], f32)
            wf = cp.tile([128, CO, C], bf)             # full Wp.T
            bp1 = cp.tile([1, C], bf)
            onesr = cp.tile([1, 128], bf)
            qT = cp.tile([128, BT], bf)
            kT = cp.tile([128, BT], bf)
            vT = cp.tile([128, BT], bf)
            vnat = cp.tile([128, B * HPC, T // KP, HD + 1], bf)
            yT = cp.tile([128, BT], bf)
            ytc = cp.tile([128, B, CO, TPB], bf)       # gathered y for own tokens
            ident = cp.tile([128, 128], bf)
            ones65 = cp.tile([HD + 1, HD], bf)
            # causal-mask factors: st[:, diag block] += Lt.T @ negI adds
            # -240 above the diagonal (exp(-240/8 + s) ~ 0), so no separate
            # mask multiply sits on the st->exp->pv dependency chain.
            lt = cp.tile([128, 128], bf)     # lt[d, p] = (d < p)
            ptd = cp.tile([128, QB], bf)     # dummy pt for the atnx variant
            negi = cp.tile([128, QB], bf)    # [-240*I | zeros]

            # DRAM bounce buffers
            bq_bounce = dp.tile([1, 3 * CW], f32)
            a2a_in0 = dp.tile([NCORE * 128, TPB], bf)
            a2a_in1 = dp.tile([NCORE * 128, TPB], bf)
            a2a_out0 = dp.tile([NCORE * 128, TPB], bf)
            a2a_out1 = dp.tile([NCORE * 128, TPB], bf)
            a2a_ins = (a2a_in0, a2a_in1)
            a2a_outs = (a2a_out0, a2a_out1)

            if not do_qkv:
                nc.gpsimd.memset(qT[:], 0.0)
                nc.gpsimd.memset(kT[:], 0.0)
                nc.gpsimd.memset(vT[:], 0.0)
            if not do_attn:
                nc.gpsimd.memset(yT[:], 0.0)
                nc.gpsimd.memset(vnat[:], 0.0)
            if not do_a2a:
                nc.gpsimd.memset(ytc[:], 0.0)
            if no_exp:
                nc.gpsimd.memset(ptd[:], 0.001)

            for _rep in range(reps):
                if n_cc:
                    # collective micro-benchmark: n_cc AllToAlls per rep
                    for i in range(n_cc):
                        nc.gpsimd.collective_compute(
                            "AllToAll", ALU.bypass, replica_groups=GROUP,
                            ins=[a2a_ins[i % 2][:].opt()],
                            outs=[a2a_outs[i % 2][:].opt()],
                        )
                    continue
                # ---- input DMAs (SP + ACT queues; earliest-needed first) ----
                nc.sync.dma_start(bq_bounce[:], bqkv_d.ap())
                nc.sync.dma_start(
                    bqkv[:],
                    bq_bounce[:].rearrange("o (p j) -> (o p) j", p=CW))
                nc.sync.dma_start(bp1[:], bp_d.ap())
                nc.scalar.dma_start(
                    wqkv[:],
                    wqkvt_d.ap().rearrange("(co p) j -> p co j", p=128))
                for bn in range(NBL):
                    eng = nc.sync if bn % 2 == 0 else nc.scalar
                    src = xt_d.ap()[:, bn * CO * QB:(bn + 1) * CO * QB]
                    eng.dma_start(xt[:, bn], src.rearrange("p (co t) -> p co t",
                                                           co=CO))
                nc.sync.dma_start(
                    wf[:], wpt_d.ap().rearrange("(co p) j -> p co j", p=128))

                # ---- const setup (Pool queue; idle until the AllToAlls) ----
                make_identity(nc, ident[:])
                nc.gpsimd.memset(ones65[:], 1.0)
                nc.gpsimd.memset(onesr[:], 1.0)
                nc.gpsimd.memset(lt[:], 1.0)
                nc.gpsimd.affine_select(
                    out=lt[:], in_=lt[:],
                    compare_op=ALU.is_ge, fill=0.0,
                    base=-1, channel_multiplier=-1,
                    pattern=[[1, 128]],
                )
                nc.gpsimd.memset(negi[:, 128:], 0.0)
                nc.scalar.activation(negi[:, 0:128], ident[:], AF.Copy,
                                     scale=-240.0)
                if do_attn:
                    nc.gpsimd.memset(vnat[:, :, :, HD:HD + 1], 1.0)

                dsts = (qT, kT, vT)

                def qkv_chain(b, n, p):
                    # one projection chunk: [128 out-ch, 512 tok]
                    tok = b * T + n * QB
                    ps = mmp.tile([128, QB], f32, tag="mm")
                    for co in range(CO):
                        nc.tensor.matmul(
                            ps[:],
                            wqkv[:, co, p * CW:(p + 1) * CW],
                            xt[:, b * NQB + n, co, :],
                            start=(co == 0),
                            stop=(co == CO - 1),
                        )
                    nc.vector.tensor_add(
                        dsts[p][:, tok:tok + QB],
                        ps[:],
                        bqkv[:, p:p + 1].to_broadcast((128, QB)),
                    )

                def vtrans_group(b, n, h):
                    # natural-layout V panels for block n, head h
                    for kc in range(n * NDIAG, (n + 1) * NDIAG):
                        tp = mmp.tile([128, HD], bf, tag="mm")
                        nc.tensor.transpose(
                            tp[:],
                            vT[HD * h:HD * (h + 1),
                               b * T + kc * KP: b * T + (kc + 1) * KP],
                            ident[HD * h:HD * (h + 1), HD * h:HD * (h + 1)],
                        )
                        nc.vector.tensor_copy(
                            vnat[:, b * HPC + h, kc, 0:HD], tp[:])

                def attn_block(b, qb, fillers, fill_every=2):
                    # depth-2 software pipeline: emit score panels (st+exp+
                    # mask) two j-steps ahead of the PV accumulations, and
                    # pull one PE filler chain every `fill_every` j-steps.
                    n_kp = (qb + 1) * NDIAG
                    q_sl = slice(b * T + qb * QB, b * T + (qb + 1) * QB)
                    ots = [otp.tile([HD + 1, QB], f32, tag="ot",
                                    name=f"ot_{b}_{qb}_{h}")
                           for h in range(HPC)]
                    pts = {}

                    def emit_st(j):
                        k_sl = slice(b * T + j * KP, b * T + (j + 1) * KP)
                        joff = j - qb * NDIAG
                        # columns < joff*KP are entirely above the causal
                        # diagonal: skip them in exp and PV
                        u0 = max(joff, 0) * KP
                        for h in range(hpc_eff):
                            hsl = slice(HD * h, HD * (h + 1))
                            st = stp.tile([128, QB], f32, tag="st")
                            nc.tensor.matmul(
                                st[:], kT[hsl, k_sl], qT[hsl, q_sl],
                                start=True, stop=(joff < 0),
                            )
                            if joff >= 0:
                                nc.tensor.matmul(
                                    st[:, u0:], lt[:], negi[:, :QB - u0],
                                    start=False, stop=True,
                                )
                            if no_exp:
                                pts[(j, h)] = (ptd, u0)
                            else:
                                pt = wp.tile([128, QB], bf, tag="pt")
                                if split_exp:
                                    mid = (u0 + QB) // 2
                                    nc.scalar.activation(
                                        pt[:, u0:mid], st[:, u0:mid],
                                        AF.Exp, scale=SCALE)
                                    nc.scalar.activation(
                                        pt[:, mid:], st[:, mid:],
                                        AF.Exp, scale=SCALE)
                                pts[(j, h)] = (pt, u0)
                                if not split_exp:
                                    nc.scalar.activation(
                                        pt[:, u0:], st[:, u0:],
                                        AF.Exp, scale=SCALE)

                    def emit_pv(j):
                        for h in range(hpc_eff):
                            pt, u0 = pts.pop((j, h))
                            if split_exp:
                                mid = (u0 + QB) // 2
                                for lo_c, hi_c in ((u0, mid), (mid, QB)):
                                    nc.tensor.matmul(
                                        ots[h][:, lo_c:hi_c],
                                        vnat[:, b * HPC + h, j, :],
                                        pt[:, lo_c:hi_c],
                                        start=(j == 0),
                                        stop=(j == n_kp - 1),
                                    )
                            else:
                                nc.tensor.matmul(
                                    ots[h][:, u0:],
                                    vnat[:, b * HPC + h, j, :],
                                    pt[:, u0:],
                                    start=(j == 0),
                                    stop=(j == n_kp - 1),
                                )

                    for j in range(n_kp + 1):
                        if j < n_kp:
                            emit_st(j)
                        if j >= 1:
                            emit_pv(j - 1)
                        if j % fill_every == 0:
                            for f in fillers:
                                f()
                                break

                    # normalize by softmax denominators (last PV row)
                    for h in range(hpc_eff):
                        rec = wp.tile([HD + 1, QB], bf, tag="rec",
                                      name=f"rec_{b}_{qb}_{h}")
                        with nc.allow_low_precision(
                                reason="bf16 denominator broadcast"):
                            nc.vector.reciprocal(
                                rec[HD:HD + 1, :], ots[h][HD:HD + 1, :])
                        ocp = wp.tile([HD, QB], f32, tag="ocp",
                                      name=f"ocp_{b}_{qb}_{h}")
                        nc.vector.tensor_copy(ocp[:], ots[h][0:HD, :])
                        # broadcast the reciprocal over 64 partitions (PE)
                        bc = otp.tile([HD, QB], f32, tag="ot",
                                      name=f"bc_{b}_{qb}_{h}")
                        nc.tensor.matmul(
                            bc[:],
                            ones65[HD:HD + 1, :],
                            rec[HD:HD + 1, :],
                            start=True, stop=True,
                        )
                        if h == 0:
                            nc.vector.tensor_mul(
                                yT[0:HD, q_sl], ocp[:], bc[:])
                        else:
                            t64 = wp.tile([HD, QB], bf, tag="t64")
                            nc.vector.tensor_mul(t64[:], ocp[:], bc[:])
                            nc.sync.dma_start(yT[HD:2 * HD, q_sl], t64[:])

                def a2a(b):
                    # yT[:, batch b] -> per-peer [128 x 256] slices ->
                    # AllToAll -> ytc[:, b] = y[all 1024 ch, own 256 tok]
                    nc.gpsimd.dma_start(
                        a2a_ins[b][:].rearrange("(d p) t -> p d t", p=128),
                        yT[:, b * T:(b + 1) * T].rearrange(
                            "p (d t) -> p d t", d=NCORE))
                    nc.gpsimd.collective_compute(
                        "AllToAll", ALU.bypass, replica_groups=GROUP,
                        ins=[a2a_ins[b][:].opt()],
                        outs=[a2a_outs[b][:].opt()],
                    )
                    nc.gpsimd.dma_start(
                        ytc[:, b],
                        a2a_outs[b][:].rearrange("(d p) t -> p d t", p=128))

                def proj_chain(b, rb):
                    # out rows [b*256 + rb*128, +128) = ytc.T @ Wp.T + bp
                    osb = wp.tile([128, C], bf, tag="osb")
                    for half in range(C // QB):
                        ps = mmp.tile([128, QB], f32, tag="mm")
                        # bias first (1-partition ones row), then accumulate
                        nc.tensor.matmul(
                            ps[:], onesr[:],
                            bp1[:, half * QB:(half + 1) * QB],
                            start=True, stop=False,
                        )
                        for co in range(CO):
                            nc.tensor.matmul(
                                ps[:],
                                ytc[:, b, co, rb * 128:(rb + 1) * 128],
                                wf[:, co, half * QB:(half + 1) * QB],
                                start=False,
                                stop=(co == CO - 1),
                            )
                        nc.vector.tensor_copy(
                            osb[:, half * QB:(half + 1) * QB], ps[:])
                    nc.sync.dma_start(
                        out_d.ap()[b * TPB + rb * 128:
                                   b * TPB + (rb + 1) * 128, :],
                        osb[:])

                # ---------------- schedule ----------------
                if do_qkv:
                    for n in range(NQB):
                        for p in range(3):
                            qkv_chain(0, n, p)
                        if do_attn:
                            for h in range(hpc_eff):
                                vtrans_group(0, n, h)

                def b1_fillers():
                    if do_qkv:
                        for n in range(NQB):
                            for p in range(3):
                                yield lambda n=n, p=p: qkv_chain(1, n, p)
                            if do_attn:
                                for h in range(hpc_eff):
                                    yield lambda n=n, h=h: vtrans_group(1, n, h)

                f1 = b1_fillers()
                if do_attn:
                    for qb in range(NQB):
                        attn_block(0, qb, f1)
                for f in f1:  # drain any leftovers
                    f()
                if do_a2a:
                    a2a(0)

                if do_attn:
                    for qb in range(NQB):
                        attn_block(1, qb, iter(()))
                if do_proj:
                    for rb in range(TPB // 128):
                        proj_chain(0, rb)
                if do_a2a:
                    a2a(1)
                if do_proj:
                    for rb in range(TPB // 128):
                        proj_chain(1, rb)

    nc.finalize()
    return nc


def _prep_in_maps(x, Wq, bq, Wk, bk, Wv, bv, Wp, bp):
    # xt[p, ((b*4+n)*8+co)*512 + t] = x[b, n*512+t, co*128+p], replicated
    x2 = x.astype(bf16).reshape(B, NQB, QB, CO, 128)
    xt = np.ascontiguousarray(x2.transpose(4, 0, 1, 3, 2))
    xt = xt.reshape(128, NBL * CO * QB)
    wpt = np.ascontiguousarray(Wp.T).astype(bf16)          # [1024, 1024]
    bp1 = np.ascontiguousarray(bp.astype(bf16).reshape(1, C))
    in_maps = []
    for i in range(NCORE):
        ch = slice(CW * i, CW * (i + 1))
        wqkvt = np.ascontiguousarray(np.concatenate(
            [Wq[ch].T, Wk[ch].T, Wv[ch].T], axis=1).astype(bf16))  # [C, 384]
        bqkv = np.stack([bq[ch], bk[ch], bv[ch]],
                        axis=1).astype(np.float32).reshape(1, 3 * CW)
        in_maps.append({
            "xt": xt,
            "wqkvt": wqkvt,
            "bqkv": np.ascontiguousarray(bqkv),
            "wpt": wpt,
            "bp": bp1,
        })
    return in_maps


def _assemble(results):
    # results[c]["out"]: [512, 1024] bf16, row b*256+i = global (b, c*256+i)
    out = np.empty((B, NCORE, TPB, C), np.float32)
    for c, r in enumerate(results):
        out[:, c] = np.asarray(r["out"]).reshape(B, TPB, C)
    return out.reshape(B, T, C)


def kernel(x, Wq, bq, Wk, bk, Wv, bv, Wp, bp):
    global _cached_nc
    x = np.asarray(x, np.float32)
    Wq, bq = np.asarray(Wq, np.float32), np.asarray(bq, np.float32)
    Wk, bk = np.asarray(Wk, np.float32), np.asarray(bk, np.float32)
    Wv, bv = np.asarray(Wv, np.float32), np.asarray(bv, np.float32)
    Wp, bp = np.asarray(Wp, np.float32), np.asarray(bp, np.float32)

    if _cached_nc is None:
        _cached_nc = _build()
    nc = _cached_nc

    in_maps = _prep_in_maps(x, Wq, bq, Wk, bk, Wv, bv, Wp, bp)
    res = run_bass_kernel_spmd(nc, in_maps, core_ids=list(range(NCORE)))
    return _assemble(res.results)


# revision 18
# speedup vs baseline: 3.1284x; 1.1061x over previous
"""Causal self-attention, 8 TRN2 cores, head-parallel, zero input collectives.

Sharding: tensor-parallel over heads (2 heads/core) with x REPLICATED per
core (staged device-side, so no on-device AllGather), and the output
projection token-parallel: after attention, one small per-batch AllToAll
redistributes yT slices ([128 ch x 256 tok] per peer) so each core projects
only its own 256 tokens per batch against the FULL Wp, with bp folded into
the matmul via a 1-partition ones row. Collectives drop from 6 (2 AllGather
+ 4 ReduceScatter, ~12.5MB) to 2 AllToAlls (0.5MB each).

Schedule: batch-0 QKV -> batch-0 attention with batch-1 QKV/V-transpose
chains interleaved between score panels (fills the PE bubbles left by the
exp latency on ACT) -> batch-0 AllToAll (Pool, overlaps batch-1 attention)
-> batch-1 attention with batch-0 projection chains as late fillers ->
batch-1 AllToAll -> batch-1 projection. Attention emits score matmuls two
panels ahead of the PV accumulation (depth-2 software pipeline).

Per-core inputs:
  xt    [128, 8*8*512] bf16  xt[p, ((b*4+n)*8+co)*512+t] = x[b, n*512+t, co*128+p]
  wqkvt [1024, 384]    bf16  [Wq_i.T | Wk_i.T | Wv_i.T] (this core's 128 ch)
  bqkv  [1, 384]       f32   (q,k,v) interleaved per channel
  wpt   [1024, 1024]   bf16  full Wp.T
  bp    [1, 1024]      bf16  full bp
Output per core:
  out   [512, 1024]    bf16  row b*256+i = global (b, c*256+i)
"""

import itertools
import sys

sys.path.insert(0, "/opt/trn_rl_repo")

import numpy as np
import ml_dtypes

import concourse.bass as bass
import concourse.mybir as mybir
import concourse.tile as tile
from concourse import bacc
from concourse.bass_utils import run_bass_kernel_spmd
from concourse.masks import make_identity

bf16 = ml_dtypes.bfloat16
B, T, C, H = 2, 2048, 1024, 16
HD = C // H              # 64
NCORE = 8
BT = B * T               # 4096
TPB = T // NCORE         # 256 tokens per core per batch (output shard)
HPC = H // NCORE         # 2 heads per core
CW = HPC * HD            # 128 channels per core
QB = 512                 # q-block width
NQB = T // QB            # 4 q-blocks per batch
KP = 128                 # k-panel width
SCALE = 1.0 / 8.0        # 1/sqrt(64)
CO = C // 128            # 8 contraction chunks
NBL = B * NQB            # 8 token blocks of 512
NDIAG = QB // KP         # 4 diagonal panels per q-block

f32 = mybir.dt.float32
bf = mybir.dt.bfloat16
AF = mybir.ActivationFunctionType
ALU = mybir.AluOpType
GROUP = [list(range(NCORE))]

_cached_nc = None


def _build(reps=1, phase="full"):
    do_qkv = phase in ("qkv", "attn", "ath1", "atnx", "atsp", "a2a", "full")
    do_attn = phase in ("attn", "ath1", "atnx", "atsp", "a2a", "full")
    do_a2a = phase in ("a2a", "full")
    do_proj = phase == "full"
    n_cc = int(phase[2:]) if phase.startswith("cc") else 0
    hpc_eff = 1 if phase == "ath1" else HPC
    no_exp = phase == "atnx"
    split_exp = phase == "atsp"

    nc = bacc.Bacc("TRN2", target_bir_lowering=False, debug=False, num_devices=NCORE)
    xt_d = nc.dram_tensor("xt", [128, NBL * CO * QB], bf, kind="ExternalInput")
    wqkvt_d = nc.dram_tensor("wqkvt", [CO * 128, 3 * CW], bf, kind="ExternalInput")
    bqkv_d = nc.dram_tensor("bqkv", [1, 3 * CW], f32, kind="ExternalInput")
    wpt_d = nc.dram_tensor("wpt", [CO * 128, C], bf, kind="ExternalInput")
    bp_d = nc.dram_tensor("bp", [1, C], bf, kind="ExternalInput")
    out_d = nc.dram_tensor("out", [B * TPB, C], bf, kind="ExternalOutput")

    with tile.TileContext(nc) as tc:
        with tc.tile_pool(name="const", bufs=1) as cp, \
             tc.tile_pool(name="dram", bufs=1, space="DRAM") as dp, \
             tc.tile_pool(name="work", bufs=5) as wp, \
             tc.tile_pool(name="mm", bufs=2, space="PSUM") as mmp, \
             tc.tile_pool(name="stp", bufs=4, space="PSUM") as stp, \
             tc.tile_pool(name="otp", bufs=2, space="PSUM") as otp:

            # ---- persistent tiles ----
            xt = cp.tile([128, NBL, CO, QB], bf)       # full x, both batches
            wqkv = cp.tile([128, CO, 3 * CW], bf)
            bqkv = cp.tile([CW, 3], f32)
            wf = cp.tile([128, CO, C], bf)             # full Wp.T
            bp1 = cp.tile([1, C], bf)
            onesr = cp.tile([1, 128], bf)
            qT = cp.tile([128, BT], bf)
            kT = cp.tile([128, BT], bf)
            vT = cp.tile([128, BT], bf)
            vnat = cp.tile([128, B * HPC, T // KP, HD + 1], bf)
            yT = cp.tile([128, BT], bf)
            ytc = cp.tile([128, B, CO, TPB], bf)       # gathered y for own tokens
            ident = cp.tile([128, 128], bf)
            ones65 = cp.tile([HD + 1, HD], bf)
            # causal-mask factors: st[:, diag block] += Lt.T @ negI adds
            # -240 above the diagonal (exp(-240/8 + s) ~ 0), so no separate
            # mask multiply sits on the st->exp->pv dependency chain.
            lt = cp.tile([128, 128], bf)     # lt[d, p] = (d < p)
            ptd = cp.tile([128, QB], bf)     # dummy pt for the atnx variant
            negi = cp.tile([128, QB], bf)    # [-240*I | zeros]

            # DRAM bounce buffers
            bq_bounce = dp.tile([1, 3 * CW], f32)
            a2a_in0 = dp.tile([NCORE * 128, TPB], bf)
            a2a_in1 = dp.tile([NCORE * 128, TPB], bf)
            a2a_out0 = dp.tile([NCORE * 128, TPB], bf)
            a2a_out1 = dp.tile([NCORE * 128, TPB], bf)
            a2a_ins = (a2a_in0, a2a_in1)
            a2a_outs = (a2a_out0, a2a_out1)

            if not do_qkv:
                nc.gpsimd.memset(qT[:], 0.0)
                nc.gpsimd.memset(kT[:], 0.0)
                nc.gpsimd.memset(vT[:], 0.0)
            if not do_attn:
                nc.gpsimd.memset(yT[:], 0.0)
                nc.gpsimd.memset(vnat[:], 0.0)
            if not do_a2a:
                nc.gpsimd.memset(ytc[:], 0.0)
            if no_exp:
                nc.gpsimd.memset(ptd[:], 0.001)

            pending = []
            for _rep in range(reps):
                if n_cc:
                    # collective micro-benchmark: n_cc AllToAlls per rep
                    for i in range(n_cc):
                        nc.gpsimd.collective_compute(
                            "AllToAll", ALU.bypass, replica_groups=GROUP,
                            ins=[a2a_ins[i % 2][:].opt()],
                            outs=[a2a_outs[i % 2][:].opt()],
                        )
                    continue
                # ---- input DMAs (SP + ACT queues; earliest-needed first) ----
                nc.sync.dma_start(bq_bounce[:], bqkv_d.ap())
                nc.sync.dma_start(
                    bqkv[:],
                    bq_bounce[:].rearrange("o (p j) -> (o p) j", p=CW))
                nc.sync.dma_start(bp1[:], bp_d.ap())
                nc.scalar.dma_start(
                    wqkv[:],
                    wqkvt_d.ap().rearrange("(co p) j -> p co j", p=128))
                for bn in range(NBL):
                    eng = nc.sync if bn % 2 == 0 else nc.scalar
                    src = xt_d.ap()[:, bn * CO * QB:(bn + 1) * CO * QB]
                    eng.dma_start(xt[:, bn], src.rearrange("p (co t) -> p co t",
                                                           co=CO))
                nc.sync.dma_start(
                    wf[:], wpt_d.ap().rearrange("(co p) j -> p co j", p=128))

                # ---- const setup (Pool queue; idle until the AllToAlls) ----
                make_identity(nc, ident[:])
                nc.gpsimd.memset(ones65[:], 1.0)
                nc.gpsimd.memset(onesr[:], 1.0)
                nc.gpsimd.memset(lt[:], 1.0)
                nc.gpsimd.affine_select(
                    out=lt[:], in_=lt[:],
                    compare_op=ALU.is_ge, fill=0.0,
                    base=-1, channel_multiplier=-1,
                    pattern=[[1, 128]],
                )
                nc.gpsimd.memset(negi[:, 128:], 0.0)
                nc.scalar.activation(negi[:, 0:128], ident[:], AF.Copy,
                                     scale=-240.0)
                if do_attn:
                    nc.gpsimd.memset(vnat[:, :, :, HD:HD + 1], 1.0)

                dsts = (qT, kT, vT)

                def qkv_chain(b, n, p):
                    # one projection chunk: [128 out-ch, 512 tok]
                    tok = b * T + n * QB
                    ps = mmp.tile([128, QB], f32, tag="mm")
                    for co in range(CO):
                        nc.tensor.matmul(
                            ps[:],
                            wqkv[:, co, p * CW:(p + 1) * CW],
                            xt[:, b * NQB + n, co, :],
                            start=(co == 0),
                            stop=(co == CO - 1),
                        )
                    nc.vector.tensor_add(
                        dsts[p][:, tok:tok + QB],
                        ps[:],
                        bqkv[:, p:p + 1].to_broadcast((128, QB)),
                    )

                def vtrans_group(b, n, h):
                    # natural-layout V panels for block n, head h
                    for kc in range(n * NDIAG, (n + 1) * NDIAG):
                        tp = mmp.tile([128, HD], bf, tag="mm")
                        nc.tensor.transpose(
                            tp[:],
                            vT[HD * h:HD * (h + 1),
                               b * T + kc * KP: b * T + (kc + 1) * KP],
                            ident[HD * h:HD * (h + 1), HD * h:HD * (h + 1)],
                        )
                        nc.vector.tensor_copy(
                            vnat[:, b * HPC + h, kc, 0:HD], tp[:])

                def attn_block(b, qb, fillers, fill_every=2):
                    # depth-2 software pipeline: emit score panels (st+exp+
                    # mask) two j-steps ahead of the PV accumulations, and
                    # pull one PE filler chain every `fill_every` j-steps.
                    n_kp = (qb + 1) * NDIAG
                    q_sl = slice(b * T + qb * QB, b * T + (qb + 1) * QB)
                    ots = [otp.tile([HD + 1, QB], f32, tag="ot",
                                    name=f"ot_{b}_{qb}_{h}")
                           for h in range(HPC)]
                    pts = {}

                    def emit_st(j):
                        k_sl = slice(b * T + j * KP, b * T + (j + 1) * KP)
                        joff = j - qb * NDIAG
                        # columns < joff*KP are entirely above the causal
                        # diagonal: skip them in exp and PV
                        u0 = max(joff, 0) * KP
                        for h in range(hpc_eff):
                            hsl = slice(HD * h, HD * (h + 1))
                            st = stp.tile([128, QB], f32, tag="st")
                            nc.tensor.matmul(
                                st[:], kT[hsl, k_sl], qT[hsl, q_sl],
                                start=True, stop=(joff < 0),
                            )
                            if joff >= 0:
                                nc.tensor.matmul(
                                    st[:, u0:], lt[:], negi[:, :QB - u0],
                                    start=False, stop=True,
                                )
                            if no_exp:
                                pts[(j, h)] = (ptd, u0)
                            else:
                                pt = wp.tile([128, QB], bf, tag="pt")
                                if split_exp:
                                    mid = (u0 + QB) // 2
                                    nc.scalar.activation(
                                        pt[:, u0:mid], st[:, u0:mid],
                                        AF.Exp, scale=SCALE)
                                    nc.scalar.activation(
                                        pt[:, mid:], st[:, mid:],
                                        AF.Exp, scale=SCALE)
                                pts[(j, h)] = (pt, u0)
                                if not split_exp:
                                    nc.scalar.activation(
                                        pt[:, u0:], st[:, u0:],
                                        AF.Exp, scale=SCALE)

                    def emit_pv(j):
                        for h in range(hpc_eff):
                            pt, u0 = pts.pop((j, h))
                            if split_exp:
                                mid = (u0 + QB) // 2
                                for lo_c, hi_c in ((u0, mid), (mid, QB)):
                                    nc.tensor.matmul(
                                        ots[h][:, lo_c:hi_c],
                                        vnat[:, b * HPC + h, j, :],
                                        pt[:, lo_c:hi_c],
                                        start=(j == 0),
                                        stop=(j == n_kp - 1),
                                    )
                            else:
                                nc.tensor.matmul(
                                    ots[h][:, u0:],
                                    vnat[:, b * HPC + h, j, :],
                                    pt[:, u0:],
                                    start=(j == 0),
                                    stop=(j == n_kp - 1),
                                )

                    for j in range(n_kp + 1):
                        if j < n_kp:
                            emit_st(j)
                        if j >= 1:
                            emit_pv(j - 1)
                        if j % fill_every == 0:
                            for f in fillers:
                                f()
                                break

                    # normalize by softmax denominators (last PV row)
                    for h in range(hpc_eff):
                        rec = wp.tile([HD + 1, QB], bf, tag="rec",
                                      name=f"rec_{b}_{qb}_{h}")
                        with nc.allow_low_precision(
                                reason="bf16 denominator broadcast"):
                            nc.vector.reciprocal(
                                rec[HD:HD + 1, :], ots[h][HD:HD + 1, :])
                        ocp = wp.tile([HD, QB], f32, tag="ocp",
                                      name=f"ocp_{b}_{qb}_{h}")
                        nc.vector.tensor_copy(ocp[:], ots[h][0:HD, :])
                        # broadcast the reciprocal over 64 partitions (PE)
                        bc = otp.tile([HD, QB], f32, tag="ot",
                                      name=f"bc_{b}_{qb}_{h}")
                        nc.tensor.matmul(
                            bc[:],
                            ones65[HD:HD + 1, :],
                            rec[HD:HD + 1, :],
                            start=True, stop=True,
                        )
                        if h == 0:
                            nc.vector.tensor_mul(
                                yT[0:HD, q_sl], ocp[:], bc[:])
                        else:
                            t64 = wp.tile([HD, QB], bf, tag="t64")
                            nc.vector.tensor_mul(t64[:], ocp[:], bc[:])
                            nc.sync.dma_start(yT[HD:2 * HD, q_sl], t64[:])

                def a2a(b):
                    # yT[:, batch b] -> per-peer [128 x 256] slices ->
                    # AllToAll -> ytc[:, b] = y[all 1024 ch, own 256 tok]
                    nc.gpsimd.dma_start(
                        a2a_ins[b][:].rearrange("(d p) t -> p d t", p=128),
                        yT[:, b * T:(b + 1) * T].rearrange(
                            "p (d t) -> p d t", d=NCORE))
                    nc.gpsimd.collective_compute(
                        "AllToAll", ALU.bypass, replica_groups=GROUP,
                        ins=[a2a_ins[b][:].opt()],
                        outs=[a2a_outs[b][:].opt()],
                    )
                    nc.gpsimd.dma_start(
                        ytc[:, b],
                        a2a_outs[b][:].rearrange("(d p) t -> p d t", p=128))

                def proj_chain(b, rb):
                    # out rows [b*256 + rb*128, +128) = ytc.T @ Wp.T + bp
                    osb = wp.tile([128, C], bf, tag="osb")
                    for half in range(C // QB):
                        ps = mmp.tile([128, QB], f32, tag="mm")
                        # bias first (1-partition ones row), then accumulate
                        nc.tensor.matmul(
                            ps[:], onesr[:],
                            bp1[:, half * QB:(half + 1) * QB],
                            start=True, stop=False,
                        )
                        for co in range(CO):
                            nc.tensor.matmul(
                                ps[:],
                                ytc[:, b, co, rb * 128:(rb + 1) * 128],
                                wf[:, co, half * QB:(half + 1) * QB],
                                start=False,
                                stop=(co == CO - 1),
                            )
                        nc.vector.tensor_copy(
                            osb[:, half * QB:(half + 1) * QB], ps[:])
                    nc.sync.dma_start(
                        out_d.ap()[b * TPB + rb * 128:
                                   b * TPB + (rb + 1) * 128, :],
                        osb[:])

                # ---------------- schedule ----------------
                # b0 QKV fused block-by-block into b0 attention; fillers =
                # previous rep's deferred b1 projection, then b1 QKV chains.
                def b1_fillers():
                    if do_qkv:
                        for n in range(NQB):
                            for p in range(3):
                                yield lambda n=n, p=p: qkv_chain(1, n, p)
                            if do_attn:
                                for h in range(hpc_eff):
                                    yield lambda n=n, h=h: vtrans_group(1, n, h)

                fillers = itertools.chain(iter(pending), b1_fillers())
                pending = []
                if do_qkv:
                    for n in range(NQB):
                        for p in range(3):
                            qkv_chain(0, n, p)
                        if do_attn:
                            for h in range(hpc_eff):
                                vtrans_group(0, n, h)
                        if do_attn:
                            attn_block(0, n, fillers)
                for f in fillers:  # drain any leftovers
                    f()
                if do_a2a:
                    a2a(0)

                if do_attn:
                    for qb in range(NQB):
                        attn_block(1, qb, iter(()))
                if do_proj:
                    for rb in range(TPB // 128):
                        proj_chain(0, rb)
                if do_a2a:
                    a2a(1)
                if do_proj:
                    if _rep == reps - 1:
                        for rb in range(TPB // 128):
                            proj_chain(1, rb)
                    else:
                        # defer the tail projection into the next rep's
                        # filler stream so the AllToAll latency overlaps
                        # the next rep's front instead of stalling PE
                        pending = [
                            (lambda rb=rb: proj_chain(1, rb))
                            for rb in range(TPB // 128)
                        ]

    nc.finalize()
    return nc


def _prep_in_maps(x, Wq, bq, Wk, bk, Wv, bv, Wp, bp):
    # xt[p, ((b*4+n)*8+co)*512 + t] = x[b, n*512+t, co*128+p], replicated
    x2 = x.astype(bf16).reshape(B, NQB, QB, CO, 128)
    xt = np.ascontiguousarray(x2.transpose(4, 0, 1, 3, 2))
    xt = xt.reshape(128, NBL * CO * QB)
    wpt = np.ascontiguousarray(Wp.T).astype(bf16)          # [1024, 1024]
    bp1 = np.ascontiguousarray(bp.astype(bf16).reshape(1, C))
    in_maps = []
    for i in range(NCORE):
        ch = slice(CW * i, CW * (i + 1))
        wqkvt = np.ascontiguousarray(np.concatenate(
            [Wq[ch].T, Wk[ch].T, Wv[ch].T], axis=1).astype(bf16))  # [C, 384]
        bqkv = np.stack([bq[ch], bk[ch], bv[ch]],
                        axis=1).astype(np.float32).reshape(1, 3 * CW)
        in_maps.append({
            "xt": xt,
            "wqkvt": wqkvt,
            "bqkv": np.ascontiguousarray(bqkv),
            "wpt": wpt,
            "bp": bp1,
        })
    return in_maps


def _assemble(results):
    # results[c]["out"]: [512, 1024] bf16, row b*256+i = global (b, c*256+i)
    out = np.empty((B, NCORE, TPB, C), np.float32)
    for c, r in enumerate(results):
        out[:, c] = np.asarray(r["out"]).reshape(B, TPB, C)
    return out.reshape(B, T, C)


def kernel(x, Wq, bq, Wk, bk, Wv, bv, Wp, bp):
    global _cached_nc
    x = np.asarray(x, np.float32)
    Wq, bq = np.asarray(Wq, np.float32), np.asarray(bq, np.float32)
    Wk, bk = np.asarray(Wk, np.float32), np.asarray(bk, np.float32)
    Wv, bv = np.asarray(Wv, np.float32), np.asarray(bv, np.float32)
    Wp, bp = np.asarray(Wp, np.float32), np.asarray(bp, np.float32)

    if _cached_nc is None:
        _cached_nc = _build()
    nc = _cached_nc

    in_maps = _prep_in_maps(x, Wq, bq, Wk, bk, Wv, bv, Wp, bp)
    res = run_bass_kernel_spmd(nc, in_maps, core_ids=list(range(NCORE)))
    return _assemble(res.results)
